# revision 11
# baseline (speedup 1.0000x reference)
"""Trainium2 Bass kernel for nn_ConvGraph_SC (gnn_message_passing).

Reference computation (per batch b of 64, N=32 nodes, C=512 channels, 7x7 spatial):
    state = input.mean(axis=(3,4))                       # [B, N, C]
    mat1  = state @ W1.T + b1
    mat2  = state @ W2.T + b2
    adj   = mat1 @ mat2.T                                # [B, N, N]
    soft  = softmax((adj - mean(adj)) / std(adj), rows)  # global mean/std, ddof=1
    out   = mean(soft @ state + state, axis=1)           # [B, C]

Device-side algebra (same as v1):
  * adj = S A S^T + su 1^T + 1 sv^T + c0, with A = W1^T W2, u = W1^T b2,
    v = W2^T b1, c0 = b1.b2 precomputed on host -> one [C,C] GEMM.
  * Row softmax is invariant to row-constant shifts -> su, c0 and the global
    mean drop out; they only enter the mean/std statistics, computed from
    per-row sums with closed-form corrections.
  * 1/std via Newton rsqrt on the vector engine (magic seed + 2 iterations).
  * out[b,c] = (1/N) sum_m (colsum(soft)[m] + 1) * state[m,c].

v2 performance restructure (the big wins):
  * Input cast to fp16 on host -> DMA bytes halved (12.8 MB/core, ~30 us at
    the 435 GB/s per-core cap) and 1 cycle/col PE matmuls.
  * Host permutes each batch to [128 part = (n, c_high), (half, s, c_low64)]
    so the 49-spatial sum is done ON THE PE: 7 accumulating identity-matmuls
    of 448 contiguous fp16 columns -> PSUM [128,(7,64)], then one DVE reduce
    of FD=448 (DVE tensor_reduce only has a 1x uop, so the old FD=3136
    reduce at 3.4 us/half was the critical path).
  * TA^T = A^T S_g^T computed per GROUP of 4 batches with 128x128 A-blocks
    as stationary (full M=128 PE rows, fp16) instead of per-batch M=32 fp32
    matmuls (~5 cyc/col in LOW_HIGH mode).

Sharding: pure data parallel, 8 batches per NeuronCore, weights replicated.
"""

import numpy as np

import concourse.bacc as bacc
import concourse.tile as tile
from concourse import masks, mybir
from concourse.bass_utils import run_bass_kernel_spmd

F32 = mybir.dt.float32
F16 = mybir.dt.float16
I32 = mybir.dt.int32
NCORES = 8
B, N, C, HW = 64, 32, 512, 49
BPC = B // NCORES          # batches per core
FREE = N * C * HW // 128   # 6272 cols per partition per batch
HALF = FREE // 2           # 3136 = 49 * 64
GROUPS = [(0, 4), (4, 3), (7, 1)]  # (start, size) stats groups

_CACHED_NC = None

A_ = mybir.AluOpType


def build_bass(debug=False):
    nc = bacc.Bacc("TRN2", target_bir_lowering=False)

    # x layout per batch: partition p = 4n + c_high (c_high = c >> 7),
    # col = 3136*h + 64*s + cl  with  c = 128*c_high + 64*h + cl, s in [0,49)
    x_d = nc.declare_dram_parameter("x", [BPC, 128, FREE], F16, isOutput=False)
    a_d = nc.declare_dram_parameter("amat", [C, C], F16, isOutput=False)
    uv_d = nc.declare_dram_parameter("uv", [C, 2], F16, isOutput=False)
    c0_d = nc.declare_dram_parameter("c0", [32, 1], F32, isOutput=False)
    out_d = nc.declare_dram_parameter("out", [128, 4 * BPC], F32, isOutput=True)

    with tile.TileContext(nc) as tc:
        with (
            nc.allow_low_precision(
                reason="fp16 intermediates; 2e-2 output tolerance"
            ),
            tc.tile_pool(name="xpool", bufs=3) as xpool,
            tc.tile_pool(name="singles", bufs=1) as singles,
            tc.tile_pool(name="srawp", bufs=3) as srawp,
            tc.tile_pool(name="tasbp", bufs=2) as tasbp,
            tc.tile_pool(name="small", bufs=2) as small,
            tc.tile_pool(name="ps_x", bufs=2, space="PSUM") as ps_x_pool,
            tc.tile_pool(name="ps_t", bufs=1, space="PSUM") as ps_t_pool,
            tc.tile_pool(name="ps_tat", bufs=1, space="PSUM") as ps_tat_pool,
            tc.tile_pool(name="ps_adj", bufs=2, space="PSUM") as ps_adj_pool,
            tc.tile_pool(name="ps_sm", bufs=2, space="PSUM") as ps_sm_pool,
        ):
            # ---- persistent tiles -----------------------------------------
            ident16 = singles.tile([128, 128], F16)
            ones_r16 = singles.tile([1, 128], F16)
            ones_r32 = singles.tile([1, 128], F32)
            ones_c32 = singles.tile([32, 1], F32)
            # A blocks: a_sb[p, 512*r + d] = A[128*r + p, d]
            a_sb = singles.tile([128, 4 * 512], F16)
            uv_sb = singles.tile([128, 8], F16)
            c0_sb = singles.tile([32, 1], F32)
            # S^T slabs: st2[j, 256*r + 32*b + k] = Ssum_b[k, 128*r + j]
            st2 = singles.tile([128, 128 * BPC], F16)
            outsb = singles.tile([128, 4 * BPC], F32)

            # constants first (gpsimd + vector, no DMA deps)
            masks.make_identity(nc, ident16[:])
            nc.vector.memset(ones_r16[:], 1.0)
            nc.vector.memset(ones_r32[:], 1.0)
            nc.vector.memset(ones_c32[:], 1.0)

            def load_weights():
                # gpsimd queue so the sync queue stays free for x halves
                for r in range(4):
                    nc.gpsimd.dma_start(
                        out=a_sb[:, 512 * r : 512 * (r + 1)],
                        in_=a_d[128 * r : 128 * (r + 1), :],
                    )
                for r in range(4):
                    nc.gpsimd.dma_start(
                        out=uv_sb[:, 2 * r : 2 * (r + 1)],
                        in_=uv_d[128 * r : 128 * (r + 1), :],
                    )
                nc.gpsimd.dma_start(out=c0_sb[:], in_=c0_d[:])

            # groups: sizes [4, 3, 1] -- last chain covers one batch only
            for bstart, gsz in GROUPS:
                # ps_sm regions (one [128, 512] fp32 bank, per group):
                #   [:1, 0:128]     colsum(soft) rows, col = 32*bp + m
                #   [:1, 128:256]   sv rows
                #   [:32, 256:260]  su as columns, col = bp
                #   [:1, 260:268]   stats cross-partition sums (S1, S2)
                #   [:32, 268:276]  stats broadcast back
                #   [:, 276:404]    weight row broadcast to 128 partitions
                ps_sm = ps_sm_pool.tile([128, 512], F32)
                ps_adj = ps_adj_pool.tile([32, 128], F32)
                gc = 32 * bstart  # st2 col offset of this group within a slab
                W = 32 * gsz

                for bp in range(gsz):
                    b = bstart + bp
                    # -- load batch (2 half-DMAs), spatial sum on PE --------
                    xb = xpool.tile([128, FREE], F16, tag="xb")
                    sraw = srawp.tile([128, 128], F16, tag="sraw")
                    for h in range(2):
                        nc.sync.dma_start(
                            out=xb[:, HALF * h : HALF * (h + 1)],
                            in_=x_d[b][:, HALF * h : HALF * (h + 1)],
                        )
                        if b == 0 and h == 0:
                            load_weights()
                        ps_xh = ps_x_pool.tile([128, 448], F32)
                        for a in range(7):
                            nc.tensor.matmul(
                                ps_xh[:],
                                ident16[:],
                                xb[:, HALF * h + 448 * a : HALF * h + 448 * (a + 1)],
                                start=(a == 0), stop=(a == 6),
                            )
                        # psum cols (s7, cl): sum the 7 s7 groups
                        nc.vector.reduce_sum(
                            out=sraw[:, 64 * h : 64 * (h + 1)],
                            in_=ps_xh[:].rearrange("p (s q) -> p q s", q=64),
                            axis=mybir.AxisListType.X,
                        )
                    ps_t = ps_t_pool.tile([128, 128], F16)
                    nc.tensor.transpose(ps_t[:], sraw[:], ident16[:])
                    # scatter: st2[j, (r, b, k)] <- ps_t[j, 4k + r]
                    nc.scalar.copy(
                        st2[:].rearrange("p (r x) -> p r x", x=256)[
                            :, :, 32 * b : 32 * (b + 1)
                        ],
                        ps_t[:].rearrange("p (k r) -> p r k", r=4),
                    )
                    # su column directly: [32,1] = S_b u at ps_sm[:32, 256+bp]
                    for r in range(4):
                        nc.tensor.matmul(
                            ps_sm[:32, 256 + bp : 257 + bp],
                            st2[:, 256 * r + 32 * b : 256 * r + 32 * (b + 1)],
                            uv_sb[:, 2 * r : 2 * r + 1],
                            start=(r == 0), stop=(r == 3),
                        )

                # -- TAT_s = sum_r A[r,s]^T @ S_g^T[r]  (whole group) -------
                ps_tat = ps_tat_pool.tile([128, 512], F32)
                for s in range(4):
                    for r in range(4):
                        nc.tensor.matmul(
                            ps_tat[:, 128 * s : 128 * s + W],
                            a_sb[:, 512 * r + 128 * s : 512 * r + 128 * (s + 1)],
                            st2[:, 256 * r + gc : 256 * r + gc + W],
                            start=(r == 0), stop=(r == 3),
                        )
                ta_sb = tasbp.tile([128, 512], F16, tag="ta_sb")
                for s in range(4):
                    nc.scalar.copy(
                        ta_sb[:, 128 * s : 128 * s + W],
                        ps_tat[:, 128 * s : 128 * s + W],
                    )

                # -- sv rows for the whole group ----------------------------
                for r in range(4):
                    nc.tensor.matmul(
                        ps_sm[:1, 128 : 128 + W],
                        uv_sb[:, 2 * r + 1 : 2 * r + 2],
                        st2[:, 256 * r + gc : 256 * r + gc + W],
                        start=(r == 0), stop=(r == 3),
                    )
                sv_sb = small.tile([1, 128], F16, tag="sv_sb")
                nc.vector.tensor_copy(sv_sb[:, 0:W], ps_sm[:1, 128 : 128 + W])

                for bp in range(gsz):
                    b = bstart + bp
                    # adjacency (minus row-constants): S A S^T + 1 sv^T
                    asl = slice(32 * bp, 32 * (bp + 1))
                    for s in range(4):
                        nc.tensor.matmul(
                            ps_adj[:, asl],
                            ta_sb[:, 128 * s + 32 * bp : 128 * s + 32 * (bp + 1)],
                            st2[:, 256 * s + 32 * b : 256 * s + 32 * (b + 1)],
                            start=(s == 0), stop=False,
                        )
                    nc.tensor.matmul(
                        ps_adj[:, asl],
                        ones_r16[0:1, 0:32],
                        sv_sb[0:1, asl],
                        start=False, stop=True,
                    )

                # ---- grouped stats: S1/S2 of TRUE adj via row sums --------
                # (cols >= gsz hold garbage; all ops are column-separated)
                q_g = small.tile([32, 4], F32, tag="q_g")
                nc.vector.tensor_scalar(
                    out=q_g[:], in0=ps_sm[:32, 256:260],
                    scalar1=c0_sb[:], scalar2=None, op0=A_.add,
                )
                t_g = small.tile([32, 4], F32, tag="t_g")
                nc.vector.reduce_sum(
                    out=t_g[:],
                    in_=ps_adj[:].rearrange("p (b m) -> p b m", m=32),
                    axis=mybir.AxisListType.X,
                )
                # rowsq: one scalar Square (fp16 out), then DVE row-sums
                sq_t = small.tile([32, 128], F16, tag="sq_t")
                nc.scalar.activation(
                    out=sq_t[:, 0:W], in_=ps_adj[:, 0:W],
                    func=mybir.ActivationFunctionType.Square,
                )
                rowsq = small.tile([32, 4], F32, tag="rowsq")
                nc.vector.reduce_sum(
                    out=rowsq[:, 0:gsz],
                    in_=sq_t[:, 0:W].rearrange("p (b m) -> p b m", m=32),
                    axis=mybir.AxisListType.X,
                )
                # stats_g: cols 0:4 = S1 rows, 4:8 = S2 rows (true adj)
                stats_g = small.tile([32, 8], F32, tag="stats_g")
                q32 = small.tile([32, 4], F32, tag="q32")
                nc.vector.tensor_scalar(
                    out=q32[:], in0=q_g[:], scalar1=32.0, scalar2=None,
                    op0=A_.mult,
                )
                nc.vector.tensor_add(stats_g[:, 0:4], q32[:], t_g[:])
                # S2row = rowsq + q*(2t + 32q); 2t + 32q = t + S1row
                h_g = small.tile([32, 4], F32, tag="h_g")
                nc.vector.tensor_add(h_g[:], t_g[:], stats_g[:, 0:4])
                s2c = small.tile([32, 4], F32, tag="s2c")
                nc.vector.tensor_mul(s2c[:], q_g[:], h_g[:])
                nc.vector.tensor_add(stats_g[:, 4:8], rowsq[:], s2c[:])

                # cross-partition sum + broadcast back (PE ones trick)
                nc.tensor.matmul(
                    ps_sm[:1, 260:268], ones_c32[:], stats_g[:],
                    start=True, stop=True,
                )
                s_sb = small.tile([1, 8], F32, tag="s_sb")
                nc.vector.tensor_copy(s_sb[:], ps_sm[:1, 260:268])
                nc.tensor.matmul(
                    ps_sm[:32, 268:276], ones_r32[0:1, 0:32], s_sb[:],
                    start=True, stop=True,
                )
                s_all = small.tile([32, 8], F32, tag="s_all")
                nc.vector.tensor_copy(s_all[:], ps_sm[:32, 268:276])

                # ---- inv_std = sqrt(1023)/sqrt(S2 - S1^2/1024) ------------
                NB = 1024.0  # adj elements per batch (stats are per batch)
                KS = float(np.sqrt(np.float64(NB - 1.0)))
                t1 = small.tile([32, 4], F32, tag="t1")
                nc.vector.tensor_mul(t1[:], s_all[:, 0:4], s_all[:, 0:4])
                nc.vector.tensor_scalar(
                    out=t1[:], in0=t1[:], scalar1=-1.0 / NB, scalar2=None,
                    op0=A_.mult,
                )
                v1023 = small.tile([32, 4], F32, tag="v1023")
                nc.vector.tensor_add(v1023[:], t1[:], s_all[:, 4:8])
                # Newton rsqrt with magic seed: bits = C - (iv >> 1).
                yint = small.tile([32, 4], I32, tag="yint")
                nc.vector.tensor_scalar(
                    out=yint[:], in0=v1023[:].bitcast(I32), scalar1=1,
                    scalar2=None, op0=A_.logical_shift_right,
                )
                nc.vector.tensor_scalar(
                    out=yint[:], in0=yint[:], scalar1=-1,
                    scalar2=0x5F3759DF, op0=A_.mult, op1=A_.add,
                )
                y = small.tile([32, 4], F32, tag="y")
                nc.vector.tensor_copy(y[:], yint[:].bitcast(F32))
                ya = small.tile([32, 4], F32, tag="ya")
                yb = small.tile([32, 4], F32, tag="yb")
                for it in range(2):
                    nc.vector.tensor_mul(ya[:], y[:], y[:])
                    nc.vector.tensor_mul(yb[:], ya[:], v1023[:])
                    last = it == 1
                    nc.vector.tensor_scalar(
                        out=ya[:], in0=yb[:],
                        scalar1=(-0.5 * KS) if last else -0.5,
                        scalar2=(1.5 * KS) if last else 1.5,
                        op0=A_.mult, op1=A_.add,
                    )
                    nc.vector.tensor_mul(y[:], y[:], ya[:])
                inv_g = y  # [32, 4] inv_std per batch column

                # ---- softmax pieces --------------------------------------
                negmax = small.tile([32, 4], F32, tag="negmax")
                nc.vector.reduce_max(
                    out=negmax[:, 0:gsz],
                    in_=ps_adj[:, 0:W].rearrange("p (b m) -> p b m", m=32),
                    axis=mybir.AxisListType.X, negate=True,
                )
                negm = small.tile([32, 4], F32, tag="negm")
                nc.vector.tensor_mul(negm[:], negmax[:], inv_g[:])
                expt = small.tile([32, 128], F16, tag="expt")
                for bp in range(gsz):
                    nc.scalar.activation(
                        out=expt[:, 32 * bp : 32 * (bp + 1)],
                        in_=ps_adj[:, 32 * bp : 32 * (bp + 1)],
                        func=mybir.ActivationFunctionType.Exp,
                        bias=negm[:, bp : bp + 1], scale=inv_g[:, bp : bp + 1],
                    )
                rowsum = small.tile([32, 4], F32, tag="rowsum")
                nc.vector.reduce_sum(
                    out=rowsum[:, 0:gsz],
                    in_=expt[:, 0:W].rearrange("p (b m) -> p b m", m=32),
                    axis=mybir.AxisListType.X,
                )
                recip = small.tile([32, 4], F16, tag="recip")
                nc.vector.reciprocal(recip[:], rowsum[:])

                # w[m] = colsum(soft) per batch: rows at ps_sm[:1, 0:128]
                for bp in range(gsz):
                    nc.tensor.matmul(
                        ps_sm[:1, 32 * bp : 32 * (bp + 1)],
                        recip[:, bp : bp + 1],
                        expt[:, 32 * bp : 32 * (bp + 1)],
                        start=True, stop=True,
                    )
                wf = small.tile([1, 128], F16, tag="wf")
                nc.vector.tensor_scalar(
                    out=wf[:, 0:W], in0=ps_sm[:1, 0:W],
                    scalar1=1.0 / (N * HW), scalar2=1.0 / (N * HW),
                    op0=A_.mult, op1=A_.add,
                )
                # broadcast the group weight row to 128 partitions (rank-1)
                nc.tensor.matmul(
                    ps_sm[:, 276 : 276 + W], ones_r16[0:1, :], wf[0:1, 0:W],
                    start=True, stop=True,
                )
                wb_sb = small.tile([128, 128], F16, tag="wb_sb")
                nc.vector.tensor_copy(wb_sb[:, 0:W], ps_sm[:, 276 : 276 + W])

                # ---- epilogue: out[c] = sum_k st2[c, k] * w[k] ------------
                for r in range(4):
                    scr = small.tile([128, 128], F16, tag="scr")
                    nc.vector.tensor_mul(
                        scr[:, 0:W], st2[:, 256 * r + gc : 256 * r + gc + W],
                        wb_sb[:, 0:W],
                    )
                    nc.vector.reduce_sum(
                        out=outsb[:, 4 * bstart + r : 4 * (bstart + gsz) : 4],
                        in_=scr[:, 0:W].rearrange("p (b m) -> p b m", m=32),
                        axis=mybir.AxisListType.X,
                    )
                nc.sync.dma_start(
                    out=out_d[:, 4 * bstart : 4 * (bstart + gsz)],
                    in_=outsb[:, 4 * bstart : 4 * (bstart + gsz)],
                )

    nc.finalize()
    return nc


def host_prep(input, W1, b1, W2, b2):
    # x: [B, N, C, 7, 7] -> per batch [128, (h, s, cl)] fp16
    #    partition p = 4n + c_high, col = 3136*h + 64*s + cl
    x16 = np.asarray(input, dtype=np.float16)
    xr = (
        x16.reshape(B, 32, 4, 2, 64, 49)
        .transpose(0, 1, 2, 3, 5, 4)
        .reshape(B, 128, FREE)
    )
    xr = np.ascontiguousarray(xr)
    w1 = np.asarray(W1, dtype=np.float64)
    w2 = np.asarray(W2, dtype=np.float64)
    b1 = np.asarray(b1, dtype=np.float64)
    b2 = np.asarray(b2, dtype=np.float64)
    amat = np.ascontiguousarray((w1.T @ w2) / (HW * HW), dtype=np.float16)
    u = (w1.T @ b2) / HW
    v = (w2.T @ b1) / HW
    uv = np.ascontiguousarray(np.stack([u, v], axis=1), dtype=np.float16)
    c0 = np.full((32, 1), float(b1 @ b2), dtype=np.float32)
    return xr, amat, uv, c0


def make_in_maps(input, W1, b1, W2, b2):
    xr, amat, uv, c0 = host_prep(input, W1, b1, W2, b2)
    in_maps = []
    for i in range(NCORES):
        shard = xr[BPC * i : BPC * (i + 1)]
        in_maps.append({"x": shard, "amat": amat, "uv": uv, "c0": c0})
    return in_maps


def kernel(input, W1, b1, W2, b2):
    global _CACHED_NC
    if _CACHED_NC is None:
        _CACHED_NC = build_bass()
    nc = _CACHED_NC

    in_maps = make_in_maps(input, W1, b1, W2, b2)
    res = run_bass_kernel_spmd(nc, in_maps, list(range(NCORES)))

    out = np.empty((B, C), dtype=np.float32)
    for i in range(NCORES):
        o = res.results[i]["out"]  # [128, 4*BPC], col = 4b + r
        out[BPC * i : BPC * (i + 1)] = (
            o.reshape(128, BPC, 4).transpose(1, 2, 0).reshape(BPC, C)
        )
    return out


# revision 12
# speedup vs baseline: 1.0595x; 1.0595x over previous
"""Trainium2 Bass kernel for nn_ConvGraph_SC (gnn_message_passing).

Reference computation (per batch b of 64, N=32 nodes, C=512 channels, 7x7 spatial):
    state = input.mean(axis=(3,4))                       # [B, N, C]
    mat1  = state @ W1.T + b1
    mat2  = state @ W2.T + b2
    adj   = mat1 @ mat2.T                                # [B, N, N]
    soft  = softmax((adj - mean(adj)) / std(adj), rows)  # global mean/std, ddof=1
    out   = mean(soft @ state + state, axis=1)           # [B, C]

Device-side algebra (same as v1):
  * adj = S A S^T + su 1^T + 1 sv^T + c0, with A = W1^T W2, u = W1^T b2,
    v = W2^T b1, c0 = b1.b2 precomputed on host -> one [C,C] GEMM.
  * Row softmax is invariant to row-constant shifts -> su, c0 and the global
    mean drop out; they only enter the mean/std statistics, computed from
    per-row sums with closed-form corrections.
  * 1/std via Newton rsqrt on the vector engine (magic seed + 2 iterations).
  * out[b,c] = (1/N) sum_m (colsum(soft)[m] + 1) * state[m,c].

v2 performance restructure (the big wins):
  * Input cast to fp16 on host -> DMA bytes halved (12.8 MB/core, ~30 us at
    the 435 GB/s per-core cap) and 1 cycle/col PE matmuls.
  * Host permutes each batch to [128 part = (n, c_high), (half, s, c_low64)]
    so the 49-spatial sum is done ON THE PE: 7 accumulating identity-matmuls
    of 448 contiguous fp16 columns -> PSUM [128,(7,64)], then one DVE reduce
    of FD=448 (DVE tensor_reduce only has a 1x uop, so the old FD=3136
    reduce at 3.4 us/half was the critical path).
  * TA^T = A^T S_g^T computed per GROUP of 4 batches with 128x128 A-blocks
    as stationary (full M=128 PE rows, fp16) instead of per-batch M=32 fp32
    matmuls (~5 cyc/col in LOW_HIGH mode).

Sharding: pure data parallel, 8 batches per NeuronCore, weights replicated.
"""

import numpy as np

import concourse.bacc as bacc
import concourse.tile as tile
from concourse import masks, mybir
from concourse.bass_utils import run_bass_kernel_spmd

F32 = mybir.dt.float32
F16 = mybir.dt.float16
I32 = mybir.dt.int32
NCORES = 8
B, N, C, HW = 64, 32, 512, 49
BPC = B // NCORES          # batches per core
FREE = N * C * HW // 128   # 6272 cols per partition per batch
HALF = FREE // 2           # 3136 = 49 * 64
GROUPS = [(0, 4), (4, 3), (7, 1)]  # (start, size) stats groups

_CACHED_NC = None

A_ = mybir.AluOpType


def build_bass(debug=False):
    nc = bacc.Bacc("TRN2", target_bir_lowering=False)

    # x layout per batch: partition p = 4n + c_high (c_high = c >> 7),
    # col = 3136*h + 64*s + cl  with  c = 128*c_high + 64*h + cl, s in [0,49)
    x_d = nc.declare_dram_parameter("x", [BPC, 128, FREE], F16, isOutput=False)
    a_d = nc.declare_dram_parameter("amat", [C, C], F16, isOutput=False)
    uv_d = nc.declare_dram_parameter("uv", [C, 2], F16, isOutput=False)
    c0_d = nc.declare_dram_parameter("c0", [32, 1], F32, isOutput=False)
    out_d = nc.declare_dram_parameter("out", [128, 4 * BPC], F32, isOutput=True)

    with tile.TileContext(nc) as tc:
        with (
            nc.allow_low_precision(
                reason="fp16 intermediates; 2e-2 output tolerance"
            ),
            tc.tile_pool(name="xpool", bufs=4) as xpool,
            tc.tile_pool(name="singles", bufs=1) as singles,
            tc.tile_pool(name="srawp", bufs=3) as srawp,
            tc.tile_pool(name="tasbp", bufs=2) as tasbp,
            tc.tile_pool(name="small", bufs=2) as small,
            tc.tile_pool(name="ps_x", bufs=2, space="PSUM") as ps_x_pool,
            tc.tile_pool(name="ps_t", bufs=1, space="PSUM") as ps_t_pool,
            tc.tile_pool(name="ps_tat", bufs=1, space="PSUM") as ps_tat_pool,
            tc.tile_pool(name="ps_adj", bufs=2, space="PSUM") as ps_adj_pool,
            tc.tile_pool(name="ps_sm", bufs=2, space="PSUM") as ps_sm_pool,
        ):
            # ---- persistent tiles -----------------------------------------
            ident16 = singles.tile([128, 128], F16)
            ones_r16 = singles.tile([1, 128], F16)
            ones_r32 = singles.tile([1, 128], F32)
            ones_c32 = singles.tile([32, 1], F32)
            # A blocks: a_sb[p, 512*r + d] = A[128*r + p, d]
            a_sb = singles.tile([128, 4 * 512], F16)
            uv_sb = singles.tile([128, 8], F16)
            c0_sb = singles.tile([32, 1], F32)
            # S^T slabs: st2[j, 256*r + 32*b + k] = Ssum_b[k, 128*r + j]
            st2 = singles.tile([128, 128 * BPC], F16)
            outsb = singles.tile([128, 4 * BPC], F32)

            # constants first (gpsimd + vector, no DMA deps)
            masks.make_identity(nc, ident16[:])
            nc.vector.memset(ones_r16[:], 1.0)
            nc.vector.memset(ones_r32[:], 1.0)
            nc.vector.memset(ones_c32[:], 1.0)

            def load_weights():
                # gpsimd queue so the sync queue stays free for x halves
                for r in range(4):
                    nc.gpsimd.dma_start(
                        out=a_sb[:, 512 * r : 512 * (r + 1)],
                        in_=a_d[128 * r : 128 * (r + 1), :],
                    )
                for r in range(4):
                    nc.gpsimd.dma_start(
                        out=uv_sb[:, 2 * r : 2 * (r + 1)],
                        in_=uv_d[128 * r : 128 * (r + 1), :],
                    )
                nc.gpsimd.dma_start(out=c0_sb[:], in_=c0_d[:])

            # -------- per-batch and per-group emitters ---------------------
            # Emission order is hand-pipelined so group chains never sit in
            # front of a later batch's xsum work in the in-order PE queue.
            gstate = {}

            def emit_batch(b):
                # load batch (2 half-DMAs), spatial sum on PE, reduce,
                # transpose, scatter into st2
                xb = xpool.tile([128, FREE], F16, tag="xb")
                sraw = srawp.tile([128, 128], F16, tag="sraw")
                for h in range(2):
                    nc.sync.dma_start(
                        out=xb[:, HALF * h : HALF * (h + 1)],
                        in_=x_d[b][:, HALF * h : HALF * (h + 1)],
                    )
                    if b == 0 and h == 0:
                        load_weights()
                    ps_xh = ps_x_pool.tile([128, 448], F32)
                    for a in range(7):
                        nc.tensor.matmul(
                            ps_xh[:],
                            ident16[:],
                            xb[:, HALF * h + 448 * a : HALF * h + 448 * (a + 1)],
                            start=(a == 0), stop=(a == 6),
                        )
                    # psum cols (s7, cl): sum the 7 s7 groups
                    nc.vector.reduce_sum(
                        out=sraw[:, 64 * h : 64 * (h + 1)],
                        in_=ps_xh[:].rearrange("p (s q) -> p q s", q=64),
                        axis=mybir.AxisListType.X,
                    )
                ps_t = ps_t_pool.tile([128, 128], F16)
                nc.tensor.transpose(ps_t[:], sraw[:], ident16[:])
                # scatter: st2[j, (r, b, k)] <- ps_t[j, 4k + r]
                nc.scalar.copy(
                    st2[:].rearrange("p (r x) -> p r x", x=256)[
                        :, :, 32 * b : 32 * (b + 1)
                    ],
                    ps_t[:].rearrange("p (k r) -> p r k", r=4),
                )

            def emit_group_mm(gi):
                bstart, gsz = GROUPS[gi]
                # ps_sm regions (one [128, 512] fp32 bank, per group):
                #   [:1, 0:128]     su rows; reused later for colsum(soft)
                #   [:1, 128:256]   sv rows
                #   [:32, 256:260]  su as columns, col = bp
                #   [:1, 260:268]   stats cross-partition sums (S1, S2)
                #   [:32, 268:276]  stats broadcast back
                #   [:, 276:404]    weight row broadcast to 128 partitions
                ps_sm = ps_sm_pool.tile([128, 512], F32)
                ps_adj = ps_adj_pool.tile([32, 128], F32)
                gc = 32 * bstart
                W = 32 * gsz

                # TAT_s = sum_r A[r,s]^T @ S_g^T[r]  (whole group)
                ps_tat = ps_tat_pool.tile([128, 512], F32)
                for sch in range(4):
                    for r in range(4):
                        nc.tensor.matmul(
                            ps_tat[:, 128 * sch : 128 * sch + W],
                            a_sb[:, 512 * r + 128 * sch : 512 * r + 128 * (sch + 1)],
                            st2[:, 256 * r + gc : 256 * r + gc + W],
                            start=(r == 0), stop=(r == 3),
                        )
                ta_sb = tasbp.tile([128, 512], F16, tag="ta_sb")
                for sch in range(4):
                    nc.scalar.copy(
                        ta_sb[:, 128 * sch : 128 * sch + W],
                        ps_tat[:, 128 * sch : 128 * sch + W],
                    )

                # sv rows (and su rows for multi-batch groups)
                for r in range(4):
                    nc.tensor.matmul(
                        ps_sm[:1, 128 : 128 + W],
                        uv_sb[:, 2 * r + 1 : 2 * r + 2],
                        st2[:, 256 * r + gc : 256 * r + gc + W],
                        start=(r == 0), stop=(r == 3),
                    )
                sv_sb = small.tile([1, 128], F16, tag="sv_sb")
                nc.vector.tensor_copy(sv_sb[:, 0:W], ps_sm[:1, 128 : 128 + W])
                if gsz > 1:
                    for r in range(4):
                        nc.tensor.matmul(
                            ps_sm[:1, 0:W],
                            uv_sb[:, 2 * r : 2 * r + 1],
                            st2[:, 256 * r + gc : 256 * r + gc + W],
                            start=(r == 0), stop=(r == 3),
                        )
                    su_sb = small.tile([1, 128], F16, tag="su_sb")
                    nc.vector.tensor_copy(su_sb[:, 0:W], ps_sm[:1, 0:W])
                    for bp in range(gsz):
                        nc.tensor.matmul(
                            ps_sm[:32, 256 + bp : 257 + bp],
                            su_sb[0:1, 32 * bp : 32 * (bp + 1)],
                            ones_r16[0:1, 0:1],
                            start=True, stop=True,
                        )
                else:
                    # single batch: 4 direct 1-col accumulating matmuls
                    for r in range(4):
                        nc.tensor.matmul(
                            ps_sm[:32, 256:257],
                            st2[:, 256 * r + gc : 256 * r + gc + 32],
                            uv_sb[:, 2 * r : 2 * r + 1],
                            start=(r == 0), stop=(r == 3),
                        )

                for bp in range(gsz):
                    b = bstart + bp
                    # adjacency (minus row-constants): S A S^T + 1 sv^T
                    asl = slice(32 * bp, 32 * (bp + 1))
                    for sch in range(4):
                        nc.tensor.matmul(
                            ps_adj[:, asl],
                            ta_sb[:, 128 * sch + 32 * bp : 128 * sch + 32 * (bp + 1)],
                            st2[:, 256 * sch + 32 * b : 256 * sch + 32 * (b + 1)],
                            start=(sch == 0), stop=False,
                        )
                    nc.tensor.matmul(
                        ps_adj[:, asl],
                        ones_r16[0:1, 0:32],
                        sv_sb[0:1, asl],
                        start=False, stop=True,
                    )
                gstate[gi] = (ps_sm, ps_adj)

            def emit_chain(gi):
                bstart, gsz = GROUPS[gi]
                ps_sm, ps_adj = gstate.pop(gi)
                gc = 32 * bstart
                W = 32 * gsz

                # grouped stats: S1/S2 of TRUE adj via row sums
                # (cols >= gsz hold garbage; all ops are column-separated)
                q_g = small.tile([32, 4], F32, tag="q_g")
                nc.vector.tensor_scalar(
                    out=q_g[:], in0=ps_sm[:32, 256:260],
                    scalar1=c0_sb[:], scalar2=None, op0=A_.add,
                )
                t_g = small.tile([32, 4], F32, tag="t_g")
                nc.vector.reduce_sum(
                    out=t_g[:],
                    in_=ps_adj[:].rearrange("p (b m) -> p b m", m=32),
                    axis=mybir.AxisListType.X,
                )
                # rowsq: one scalar Square (fp16 out), then DVE row-sums
                sq_t = small.tile([32, 128], F16, tag="sq_t")
                nc.scalar.activation(
                    out=sq_t[:, 0:W], in_=ps_adj[:, 0:W],
                    func=mybir.ActivationFunctionType.Square,
                )
                rowsq = small.tile([32, 4], F32, tag="rowsq")
                nc.vector.reduce_sum(
                    out=rowsq[:, 0:gsz],
                    in_=sq_t[:, 0:W].rearrange("p (b m) -> p b m", m=32),
                    axis=mybir.AxisListType.X,
                )
                # stats_g: cols 0:4 = S1 rows, 4:8 = S2 rows (true adj)
                stats_g = small.tile([32, 8], F32, tag="stats_g")
                q32 = small.tile([32, 4], F32, tag="q32")
                nc.vector.tensor_scalar(
                    out=q32[:], in0=q_g[:], scalar1=32.0, scalar2=None,
                    op0=A_.mult,
                )
                nc.vector.tensor_add(stats_g[:, 0:4], q32[:], t_g[:])
                # S2row = rowsq + q*(2t + 32q); 2t + 32q = t + S1row
                h_g = small.tile([32, 4], F32, tag="h_g")
                nc.vector.tensor_add(h_g[:], t_g[:], stats_g[:, 0:4])
                s2c = small.tile([32, 4], F32, tag="s2c")
                nc.vector.tensor_mul(s2c[:], q_g[:], h_g[:])
                nc.vector.tensor_add(stats_g[:, 4:8], rowsq[:], s2c[:])

                # cross-partition sum + broadcast back (PE ones trick)
                nc.tensor.matmul(
                    ps_sm[:1, 260:268], ones_c32[:], stats_g[:],
                    start=True, stop=True,
                )
                s_sb = small.tile([1, 8], F32, tag="s_sb")
                nc.vector.tensor_copy(s_sb[:], ps_sm[:1, 260:268])
                nc.tensor.matmul(
                    ps_sm[:32, 268:276], ones_r32[0:1, 0:32], s_sb[:],
                    start=True, stop=True,
                )
                s_all = small.tile([32, 8], F32, tag="s_all")
                nc.vector.tensor_copy(s_all[:], ps_sm[:32, 268:276])

                # inv_std = sqrt(1023)/sqrt(S2 - S1^2/1024)
                NB = 1024.0  # adj elements per batch (stats are per batch)
                KS = float(np.sqrt(np.float64(NB - 1.0)))
                t1 = small.tile([32, 4], F32, tag="t1")
                nc.vector.tensor_mul(t1[:], s_all[:, 0:4], s_all[:, 0:4])
                nc.vector.tensor_scalar(
                    out=t1[:], in0=t1[:], scalar1=-1.0 / NB, scalar2=None,
                    op0=A_.mult,
                )
                v1023 = small.tile([32, 4], F32, tag="v1023")
                nc.vector.tensor_add(v1023[:], t1[:], s_all[:, 4:8])
                # Newton rsqrt with magic seed: bits = C - (iv >> 1).
                yint = small.tile([32, 4], I32, tag="yint")
                nc.vector.tensor_scalar(
                    out=yint[:], in0=v1023[:].bitcast(I32), scalar1=1,
                    scalar2=None, op0=A_.logical_shift_right,
                )
                nc.vector.tensor_scalar(
                    out=yint[:], in0=yint[:], scalar1=-1,
                    scalar2=0x5F3759DF, op0=A_.mult, op1=A_.add,
                )
                y = small.tile([32, 4], F32, tag="y")
                nc.vector.tensor_copy(y[:], yint[:].bitcast(F32))
                ya = small.tile([32, 4], F32, tag="ya")
                yb = small.tile([32, 4], F32, tag="yb")
                for it in range(2):
                    nc.vector.tensor_mul(ya[:], y[:], y[:])
                    nc.vector.tensor_mul(yb[:], ya[:], v1023[:])
                    last = it == 1
                    nc.vector.tensor_scalar(
                        out=ya[:], in0=yb[:],
                        scalar1=(-0.5 * KS) if last else -0.5,
                        scalar2=(1.5 * KS) if last else 1.5,
                        op0=A_.mult, op1=A_.add,
                    )
                    nc.vector.tensor_mul(y[:], y[:], ya[:])
                inv_g = y  # [32, 4] inv_std per batch column

                # softmax pieces
                negmax = small.tile([32, 4], F32, tag="negmax")
                nc.vector.reduce_max(
                    out=negmax[:, 0:gsz],
                    in_=ps_adj[:, 0:W].rearrange("p (b m) -> p b m", m=32),
                    axis=mybir.AxisListType.X, negate=True,
                )
                negm = small.tile([32, 4], F32, tag="negm")
                nc.vector.tensor_mul(negm[:], negmax[:], inv_g[:])
                expt = small.tile([32, 128], F16, tag="expt")
                for bp in range(gsz):
                    nc.scalar.activation(
                        out=expt[:, 32 * bp : 32 * (bp + 1)],
                        in_=ps_adj[:, 32 * bp : 32 * (bp + 1)],
                        func=mybir.ActivationFunctionType.Exp,
                        bias=negm[:, bp : bp + 1], scale=inv_g[:, bp : bp + 1],
                    )
                rowsum = small.tile([32, 4], F32, tag="rowsum")
                nc.vector.reduce_sum(
                    out=rowsum[:, 0:gsz],
                    in_=expt[:, 0:W].rearrange("p (b m) -> p b m", m=32),
                    axis=mybir.AxisListType.X,
                )
                recip = small.tile([32, 4], F16, tag="recip")
                nc.vector.reciprocal(recip[:], rowsum[:])

                # w[m] = colsum(soft) per batch: rows at ps_sm[:1, 0:128]
                for bp in range(gsz):
                    nc.tensor.matmul(
                        ps_sm[:1, 32 * bp : 32 * (bp + 1)],
                        recip[:, bp : bp + 1],
                        expt[:, 32 * bp : 32 * (bp + 1)],
                        start=True, stop=True,
                    )
                wf = small.tile([1, 128], F16, tag="wf")
                nc.vector.tensor_scalar(
                    out=wf[:, 0:W], in0=ps_sm[:1, 0:W],
                    scalar1=1.0 / (N * HW), scalar2=1.0 / (N * HW),
                    op0=A_.mult, op1=A_.add,
                )
                # broadcast the group weight row to 128 partitions (rank-1)
                nc.tensor.matmul(
                    ps_sm[:, 276 : 276 + W], ones_r16[0:1, :], wf[0:1, 0:W],
                    start=True, stop=True,
                )
                wb_sb = small.tile([128, 128], F16, tag="wb_sb")
                nc.vector.tensor_copy(wb_sb[:, 0:W], ps_sm[:, 276 : 276 + W])

                # epilogue: out[c] = sum_k st2[c, k] * w[k]
                for r in range(4):
                    scr = small.tile([128, 128], F16, tag="scr")
                    nc.vector.tensor_mul(
                        scr[:, 0:W], st2[:, 256 * r + gc : 256 * r + gc + W],
                        wb_sb[:, 0:W],
                    )
                    nc.vector.reduce_sum(
                        out=outsb[:, 4 * bstart + r : 4 * (bstart + gsz) : 4],
                        in_=scr[:, 0:W].rearrange("p (b m) -> p b m", m=32),
                        axis=mybir.AxisListType.X,
                    )
                nc.sync.dma_start(
                    out=out_d[:, 4 * bstart : 4 * (bstart + gsz)],
                    in_=outsb[:, 4 * bstart : 4 * (bstart + gsz)],
                )

            # -------- hand-pipelined schedule ------------------------------
            for b in range(4):
                emit_batch(b)
            emit_group_mm(0)
            emit_batch(4)
            emit_chain(0)
            emit_batch(5)
            emit_batch(6)
            emit_group_mm(1)
            emit_batch(7)
            emit_chain(1)
            emit_group_mm(2)
            emit_chain(2)

    nc.finalize()
    return nc


def host_prep(input, W1, b1, W2, b2):
    # x: [B, N, C, 7, 7] -> per batch [128, (h, s, cl)] fp16
    #    partition p = 4n + c_high, col = 3136*h + 64*s + cl
    x16 = np.asarray(input, dtype=np.float16)
    xr = (
        x16.reshape(B, 32, 4, 2, 64, 49)
        .transpose(0, 1, 2, 3, 5, 4)
        .reshape(B, 128, FREE)
    )
    xr = np.ascontiguousarray(xr)
    w1 = np.asarray(W1, dtype=np.float64)
    w2 = np.asarray(W2, dtype=np.float64)
    b1 = np.asarray(b1, dtype=np.float64)
    b2 = np.asarray(b2, dtype=np.float64)
    amat = np.ascontiguousarray((w1.T @ w2) / (HW * HW), dtype=np.float16)
    u = (w1.T @ b2) / HW
    v = (w2.T @ b1) / HW
    uv = np.ascontiguousarray(np.stack([u, v], axis=1), dtype=np.float16)
    c0 = np.full((32, 1), float(b1 @ b2), dtype=np.float32)
    return xr, amat, uv, c0


def make_in_maps(input, W1, b1, W2, b2):
    xr, amat, uv, c0 = host_prep(input, W1, b1, W2, b2)
    in_maps = []
    for i in range(NCORES):
        shard = xr[BPC * i : BPC * (i + 1)]
        in_maps.append({"x": shard, "amat": amat, "uv": uv, "c0": c0})
    return in_maps


def kernel(input, W1, b1, W2, b2):
    global _CACHED_NC
    if _CACHED_NC is None:
        _CACHED_NC = build_bass()
    nc = _CACHED_NC

    in_maps = make_in_maps(input, W1, b1, W2, b2)
    res = run_bass_kernel_spmd(nc, in_maps, list(range(NCORES)))

    out = np.empty((B, C), dtype=np.float32)
    for i in range(NCORES):
        o = res.results[i]["out"]  # [128, 4*BPC], col = 4b + r
        out[BPC * i : BPC * (i + 1)] = (
            o.reshape(128, BPC, 4).transpose(1, 2, 0).reshape(BPC, C)
        )
    return out
